# revision 26
# baseline (speedup 1.0000x reference)
"""Trainium2 Bass kernel for a residual MLP with training-mode BatchNorm.

Net: x[4194304, 5] -> 4 residual blocks of (Lin5x5 -> BN -> ReLU -> Lin5x5 ->
BN -> +skip(Lin5x5) -> ReLU) -> Lin(5->2) -> BN -> ReLU -> Lin(2->1).

Strategy (pure data parallel over 8 cores, batch sharded):
- Each core keeps its 524288-row shard SBUF-resident in a transposed
  block-diagonal layout: partitions 5g+k (g in 0..24 groups, k feature),
  free dim = rows. All 10 linears are PE matmuls against host-prebuilt
  block-diagonal weights (lhsT[5g+k, 5g+j] = W[j, k]).
- Linear bias before a BN cancels (BN subtracts the mean), so matmuls are
  bias-free; BN becomes y -> alpha*y + beta applied by the ScalarE
  activation (per-partition scale/bias APs) fused with ReLU, where
  alpha = g/sqrt(var+eps), beta = b - alpha*mu from GLOBAL batch stats.
- Global stats: per 512-col chunk DVE bn_stats on the matmul PSUM output,
  bn_aggr, convert to (sum, sumsq), group-reduce via a selector matmul,
  AllReduce of [5, 2] floats over the 8 cores, tiny post-math on device.
- Residual add: the skip matmul accumulates into the same PSUM group with
  weights pre-divided by alpha (column-scaled on device) plus a ones-row
  bias trick, so block end stays a single activation op.
"""

import numpy as np

import concourse.bass as bass
import concourse.bacc as bacc
import concourse.tile as tile
import concourse.mybir as mybir
from concourse.bass_utils import run_bass_kernel_spmd

f32 = mybir.dt.float32
Alu = mybir.AluOpType
Act = mybir.ActivationFunctionType

N_CORES = 8
N_TOTAL = 4194304
R = N_TOTAL // N_CORES   # 524288 rows per core
PA = R // 128            # 4096 rows per partition stripe
G = 25                   # row groups on partitions (25*5 = 125 partitions)
NCH = (PA + G - 1) // G  # 164 transpose chunks of 128 cols
COLS = NCH * 128         # 20992 free columns in the resident buffers
MMW = 512
NMM = COLS // MMW        # 41 matmul chunks
NFULL = PA // G          # 163 full transpose chunks
LASTG = PA - NFULL * G   # 21 real groups in the last chunk
EPS = 1e-5
STAGE_A = 100            # rows-per-partition per staging DMA (4 chunks)
NSTAGE = NCH // 4        # 41 staging tiles


def _build_program(reps=1, no_collective=False, n_cores=N_CORES):
    nc = bacc.Bacc("TRN2", target_bir_lowering=False, debug=False,
                   num_devices=n_cores)

    x_in = nc.dram_tensor("x_shard", [R, 5], f32, kind="ExternalInput")
    wlin_d = nc.dram_tensor("wlin", [8, 125, 125], f32, kind="ExternalInput")
    wskip_d = nc.dram_tensor("wskip", [4, 125, 125], f32, kind="ExternalInput")
    w9_d = nc.dram_tensor("w9", [125, 50], f32, kind="ExternalInput")
    w10t_d = nc.dram_tensor("w10t", [50, 25], f32, kind="ExternalInput")
    ident_d = nc.dram_tensor("ident", [128, 128], f32, kind="ExternalInput")
    sel_d = nc.dram_tensor("sel", [125, 5], f32, kind="ExternalInput")
    sel2_d = nc.dram_tensor("sel2", [5, 125], f32, kind="ExternalInput")
    sel9_d = nc.dram_tensor("sel9", [50, 2], f32, kind="ExternalInput")
    sel92_d = nc.dram_tensor("sel92", [2, 50], f32, kind="ExternalInput")
    onesrow_d = nc.dram_tensor("onesrow", [1, 125], f32, kind="ExternalInput")
    pmask_d = nc.dram_tensor("pmask", [125, 1], f32, kind="ExternalInput")
    pmask9_d = nc.dram_tensor("pmask9", [50, 1], f32, kind="ExternalInput")
    gmat_d = nc.dram_tensor("gmat", [5, 8], f32, kind="ExternalInput")
    bmat_d = nc.dram_tensor("bmat", [5, 8], f32, kind="ExternalInput")
    g9_d = nc.dram_tensor("g9", [2, 1], f32, kind="ExternalInput")
    b9_d = nc.dram_tensor("b9", [2, 1], f32, kind="ExternalInput")
    bs125_d = nc.dram_tensor("bs125", [125, 4], f32, kind="ExternalInput")
    b10_d = nc.dram_tensor("b10rep", [128, 1], f32, kind="ExternalInput")
    out_d = nc.dram_tensor("out", [R, 1], f32, kind="ExternalOutput")

    x_re = x_in.ap().rearrange("(p a) k -> p (a k)", p=128)
    out_re = out_d.ap().rearrange("(p a) o -> p (a o)", p=128)

    with tile.TileContext(nc) as tc:
        with (
            tc.tile_pool(name="persist", bufs=1) as pp,
            tc.tile_pool(name="stage", bufs=2) as stp,
            tc.tile_pool(name="ob", bufs=2) as obp,
            tc.tile_pool(name="small", bufs=2) as sp,
            tc.tile_pool(name="dram", bufs=2, space="DRAM") as dram,
            tc.tile_pool(name="mmps", bufs=4, space="PSUM") as mmps,
            tc.tile_pool(name="smps", bufs=2, space="PSUM") as smps,
        ):
            # ---- persistent SBUF state ----
            Z = pp.tile([125, COLS], f32, tag="Zbuf")
            B = pp.tile([125, COLS], f32, tag="Bbuf")
            wl = [pp.tile([125, 125], f32, tag=f"wl{i}", name=f"wl{i}")
                  for i in range(8)]
            ws = [pp.tile([125, 125], f32, tag=f"ws{b}", name=f"ws{b}")
                  for b in range(4)]
            w9 = pp.tile([125, 50], f32, tag="w9")
            w10t = pp.tile([50, 25], f32, tag="w10t")
            ident = pp.tile([128, 128], f32, tag="ident")
            sel = pp.tile([125, 5], f32, tag="sel")
            sel2 = pp.tile([5, 125], f32, tag="sel2")
            sel9 = pp.tile([50, 2], f32, tag="sel9")
            sel92 = pp.tile([2, 50], f32, tag="sel92")
            onesrow = pp.tile([1, 125], f32, tag="onesrow")
            gmat = pp.tile([5, 8], f32, tag="gmat")
            bmat = pp.tile([5, 8], f32, tag="bmat")
            g9 = pp.tile([2, 1], f32, tag="g9")
            b9 = pp.tile([2, 1], f32, tag="b9")
            bs125 = pp.tile([125, 4], f32, tag="bs125")
            b10 = pp.tile([128, 1], f32, tag="b10")
            pmask = pp.tile([125, 1], f32, tag="pmask")
            pmask9 = pp.tile([50, 1], f32, tag="pmask9")
            epsc = pp.tile([5, 1], f32, tag="epsc")
            nc.vector.memset(epsc[:], EPS)

            for t, d in [(w9, w9_d), (w10t, w10t_d), (ident, ident_d),
                         (sel, sel_d), (sel2, sel2_d), (sel9, sel9_d),
                         (sel92, sel92_d), (onesrow, onesrow_d),
                         (gmat, gmat_d), (bmat, bmat_d), (g9, g9_d),
                         (b9, b9_d), (bs125, bs125_d), (b10, b10_d),
                         (pmask, pmask_d), (pmask9, pmask9_d)]:
                nc.sync.dma_start(t[:], d.ap())
            for i in range(8):
                nc.sync.dma_start(wl[i][:], wlin_d.ap()[i, :, :])
            for b in range(4):
                nc.sync.dma_start(ws[b][:], wskip_d.ap()[b, :, :])

            # ---- P0: load x and transpose into block-diagonal layout ----
            def load_phase():
                for s in range(NSTAGE):
                    a0 = s * STAGE_A
                    awidth = min(STAGE_A, PA - a0)
                    st = stp.tile([128, STAGE_A * 5], f32, tag="stage",
                                  name="st")
                    nc.sync.dma_start(st[:, : awidth * 5],
                                      x_re[:, a0 * 5: (a0 + awidth) * 5])
                    tp = mmps.tile([125, 512], f32, tag="ps", name="tp")
                    for j in range(4):
                        c = 4 * s + j
                        w = 125 if c < NFULL else LASTG * 5
                        if c >= NFULL:
                            # zero pad rows first (32-aligned part. start)
                            nc.vector.memset(
                                tp[96:125, j * 128:(j + 1) * 128], 0.0)
                        nc.tensor.transpose(tp[0:w, j * 128:(j + 1) * 128],
                                            st[:, j * 125: j * 125 + w],
                                            ident[0:128, 0:128])
                    nc.scalar.copy(Z[0:125, s * 512:(s + 1) * 512],
                                   tp[0:125, :])

            # ---- the 9 BN layers ----
            def bn_layer(L):
                # layer wiring
                if L == 8:
                    src, dst = Z, B
                    wraw, npo, nf = w9, 50, 2
                    selA, selB = sel9, sel92
                    gv, bv = g9[0:2, 0:1], b9[0:2, 0:1]
                    block_b = None
                elif L % 2 == 0:
                    src, dst = Z, B
                    wraw, npo, nf = wl[L], 125, 5
                    selA, selB = sel, sel2
                    gv, bv = gmat[:, L:L + 1], bmat[:, L:L + 1]
                    block_b = None
                else:
                    src, dst = B, Z
                    wraw, npo, nf = wl[L], 125, 5
                    selA, selB = sel, sel2
                    gv, bv = gmat[:, L:L + 1], bmat[:, L:L + 1]
                    block_b = L // 2

                # --- stats pass: y = Wraw @ src, accumulate bn stats ---
                stat6 = sp.tile([125, NMM * 6], f32, tag="stat6")
                for c in range(NMM):
                    sl = slice(c * MMW, (c + 1) * MMW)
                    ps = mmps.tile([125, MMW], f32, tag="ps")
                    nc.tensor.matmul(ps[0:npo, :], wraw[0:125, 0:npo],
                                     src[0:125, sl], start=True, stop=True)
                    nc.vector.bn_stats(stat6[0:npo, c * 6:(c + 1) * 6],
                                       ps[0:npo, :])
                mv = sp.tile([125, 2], f32, tag="mv")
                nc.vector.bn_aggr(mv[0:npo, :], stat6[0:npo, :])
                # sums[:,0] = mean*CNT ; sums[:,1] = (var + mean^2)*CNT
                sums = sp.tile([125, 2], f32, tag="sums")
                nc.vector.tensor_scalar(sums[0:npo, 0:1], mv[0:npo, 0:1],
                                        float(COLS), None, Alu.mult)
                t1 = sp.tile([125, 1], f32, tag="t1")
                nc.vector.tensor_tensor(t1[0:npo, :], mv[0:npo, 0:1],
                                        mv[0:npo, 0:1], Alu.mult)
                nc.vector.tensor_tensor(t1[0:npo, :], t1[0:npo, :],
                                        mv[0:npo, 1:2], Alu.add)
                nc.vector.tensor_scalar(sums[0:npo, 1:2], t1[0:npo, :],
                                        float(COLS), None, Alu.mult)
                # group-reduce 25 groups -> per-feature partial sums
                ps2 = smps.tile([125, 128], f32, tag="sp")
                nc.tensor.matmul(ps2[0:nf, 0:2], selA[0:npo, 0:nf],
                                 sums[0:npo, :], start=True, stop=True)
                arin = sp.tile([5, 2], f32, tag="arin")
                nc.scalar.copy(arin[0:nf, :], ps2[0:nf, 0:2])

                # --- AllReduce of [nf, 2] over the 8 cores ---
                d_in = dram.tile([5, 2], f32, tag="arin_d")
                d_out = dram.tile([5, 2], f32, tag="arout_d")
                nc.sync.dma_start(d_in[0:nf, :], arin[0:nf, :])
                if no_collective:
                    nc.sync.dma_start(d_out[0:nf, :], d_in[0:nf, :])
                else:
                    nc.gpsimd.collective_compute(
                        "AllReduce", Alu.add,
                        replica_groups=[list(range(n_cores))],
                        ins=[d_in[0:nf, :].opt()],
                        outs=[d_out[0:nf, :].opt()],
                    )
                gs = sp.tile([5, 2], f32, tag="gs")
                nc.sync.dma_start(gs[0:nf, :], d_out[0:nf, :])

                # --- alpha/beta from global sums ---
                ms = sp.tile([5, 2], f32, tag="ms")
                nc.vector.tensor_scalar(ms[0:nf, :], gs[0:nf, :],
                                        1.0 / N_TOTAL, None, Alu.mult)
                v = sp.tile([5, 1], f32, tag="v")
                nc.vector.tensor_tensor(v[0:nf, :], ms[0:nf, 0:1],
                                        ms[0:nf, 0:1], Alu.mult)
                nc.vector.tensor_tensor(v[0:nf, :], ms[0:nf, 1:2],
                                        v[0:nf, :], Alu.subtract)
                sd = sp.tile([5, 1], f32, tag="sd")
                nc.scalar.activation(sd[0:nf, :], v[0:nf, :], Act.Sqrt,
                                     bias=epsc[0:nf, :])
                rcp = sp.tile([5, 1], f32, tag="rcp")
                nc.vector.reciprocal(rcp[0:nf, :], sd[0:nf, :])
                ab5 = sp.tile([5, 2], f32, tag="ab5")
                nc.vector.tensor_tensor(ab5[0:nf, 0:1], rcp[0:nf, :], gv,
                                        Alu.mult)
                t2 = sp.tile([5, 1], f32, tag="t2")
                nc.vector.tensor_tensor(t2[0:nf, :], ab5[0:nf, 0:1],
                                        ms[0:nf, 0:1], Alu.mult)
                nc.vector.tensor_tensor(ab5[0:nf, 1:2], bv, t2[0:nf, :],
                                        Alu.subtract)
                # broadcast to [npo, 2]
                ps3 = smps.tile([125, 128], f32, tag="sp")
                nc.tensor.matmul(ps3[0:npo, 0:2], selB[0:nf, 0:npo],
                                 ab5[0:nf, :], start=True, stop=True)
                abf = sp.tile([125, 2], f32, tag="abf")
                nc.scalar.copy(abf[0:npo, :], ps3[0:npo, 0:2])
                alpha = abf[0:npo, 0:1]
                beta = abf[0:npo, 1:2]
                # masked variants so the padded tail rows stay exactly zero
                pm = pmask9[0:npo, 0:1] if L == 8 else pmask[0:npo, 0:1]
                abz = sp.tile([125, 2], f32, tag="abz")
                nc.vector.tensor_tensor(abz[0:npo, 0:1], abf[0:npo, 0:1],
                                        pm, Alu.mult)
                nc.vector.tensor_tensor(abz[0:npo, 1:2], abf[0:npo, 1:2],
                                        pm, Alu.mult)
                alphaz = abz[0:npo, 0:1]
                betaz = abz[0:npo, 1:2]

                # --- transform pass ---
                if block_b is None:
                    for c in range(NMM):
                        sl = slice(c * MMW, (c + 1) * MMW)
                        ps = mmps.tile([125, MMW], f32, tag="ps")
                        nc.tensor.matmul(ps[0:npo, :], wraw[0:125, 0:npo],
                                         src[0:125, sl], start=True, stop=True)
                        if c < NMM - 1:
                            nc.scalar.activation(dst[0:npo, sl], ps[0:npo, :],
                                                 Act.Relu, bias=beta,
                                                 scale=alpha)
                        else:
                            # last 128 cols hold the pad rows: masked bias
                            nc.scalar.activation(
                                dst[0:npo, c * MMW: COLS - 128],
                                ps[0:npo, 0: MMW - 128],
                                Act.Relu, bias=beta, scale=alpha)
                            nc.scalar.activation(
                                dst[0:npo, COLS - 128: COLS],
                                ps[0:npo, MMW - 128: MMW],
                                Act.Relu, bias=betaz, scale=alpha)
                else:
                    b = block_b
                    # wsaug = ws[b] * (1/alpha) per column, bias row via mm3
                    arcp = sp.tile([125, 1], f32, tag="arcp")
                    nc.vector.reciprocal(arcp[0:125, :], alpha)
                    psT = smps.tile([125, 128], f32, tag="sp")
                    nc.tensor.transpose(psT[0:1, 0:125], arcp[0:125, 0:1],
                                        ident[0:125, 0:125])
                    arT = sp.tile([1, 125], f32, tag="arT")
                    nc.scalar.copy(arT[:], psT[0:1, 0:125])
                    psR = smps.tile([125, 128], f32, tag="sp")
                    nc.tensor.matmul(psR[0:125, 0:125], onesrow[0:1, 0:125],
                                     arT[0:1, :], start=True, stop=True)
                    arep = sp.tile([125, 125], f32, tag="arep")
                    nc.scalar.copy(arep[:], psR[0:125, 0:125])
                    wsaug = sp.tile([125, 125], f32, tag="wsaug")
                    nc.vector.tensor_tensor(wsaug[:], ws[b][:],
                                            arep[:], Alu.mult)
                    # block-end bias (applied after the ACT scale): beta + bs
                    bbv = sp.tile([125, 2], f32, tag="bbv")
                    nc.vector.tensor_tensor(bbv[:, 0:1], beta,
                                            bs125[:, b:b + 1], Alu.add)
                    nc.vector.tensor_tensor(bbv[:, 1:2], bbv[:, 0:1],
                                            pmask[0:125, 0:1], Alu.mult)
                    for c in range(NMM):
                        sl = slice(c * MMW, (c + 1) * MMW)
                        ps = mmps.tile([125, MMW], f32, tag="ps")
                        nc.tensor.matmul(ps[0:125, :], wraw[0:125, 0:125],
                                         src[0:125, sl], start=True,
                                         stop=False)
                        nc.tensor.matmul(ps[0:125, :], wsaug[0:125, :],
                                         Z[0:125, sl], start=False, stop=True)
                        if c < NMM - 1:
                            nc.scalar.activation(dst[0:125, sl], ps[0:125, :],
                                                 Act.Relu, scale=alpha,
                                                 bias=bbv[:, 0:1])
                        else:
                            nc.scalar.activation(
                                dst[0:125, c * MMW: COLS - 128],
                                ps[0:125, 0: MMW - 128],
                                Act.Relu, scale=alpha, bias=bbv[:, 0:1])
                            nc.scalar.activation(
                                dst[0:125, COLS - 128: COLS],
                                ps[0:125, MMW - 128: MMW],
                                Act.Relu, scale=alphaz, bias=bbv[:, 1:2])

            # ---- final pass: y10 = z9 @ W10^T + b10, back to row-major ----
            def final_phase():
                for u in range(6):
                    t_lo = u * 8
                    t_hi = min(t_lo + 8, NMM)
                    width = 0
                    ob = obp.tile([128, 800], f32, tag="ob", name="ob")
                    for t in range(t_lo, t_hi):
                        fpw = 96 if t == NMM - 1 else 100
                        fp = mmps.tile([128, 100], f32, tag="ps", name="fp")
                        for j in range(4):
                            c = 4 * t + j
                            w = 25 if c < NFULL else LASTG  # last chunk: 21
                            nc.tensor.matmul(fp[0:128, j * 25: j * 25 + w],
                                             B[0:50, c * 128:(c + 1) * 128],
                                             w10t[0:50, 0:w],
                                             start=True, stop=True)
                        off = (t - t_lo) * 100
                        nc.scalar.activation(ob[:, off: off + fpw],
                                             fp[0:128, 0:fpw], Act.Identity,
                                             bias=b10[:, 0:1])
                        width += fpw
                    nc.sync.dma_start(out_re[:, t_lo * 100: t_lo * 100 +
                                             width], ob[:, 0:width])

            for _rep in range(reps):
                load_phase()
                for L in range(9):
                    bn_layer(L)
                final_phase()

    nc.compile()
    return nc


def _blockdiag(w, groups):
    # w: [out_f, in_f] -> lhsT[g*in_f + k, g*out_f + j] = w[j, k]
    in_f, out_f = w.shape[1], w.shape[0]
    m = np.zeros((groups * in_f, groups * out_f), np.float32)
    for g in range(groups):
        m[g * in_f:(g + 1) * in_f, g * out_f:(g + 1) * out_f] = w.T
    return m


_PROGRAM_CACHE = {}


def _get_program(reps=1):
    key = ("nc", reps)
    if key not in _PROGRAM_CACHE:
        _PROGRAM_CACHE[key] = _build_program(reps=reps)
    return _PROGRAM_CACHE[key]


def make_in_maps(x, lins_w, lins_b, skips_w, skips_b, bn_g, bn_b,
                 lin9_w, lin9_b, bn9_g, bn9_b, lin10_w, lin10_b, **_unused):
    x = np.ascontiguousarray(np.asarray(x, np.float32))
    lins_w = np.asarray(lins_w, np.float32)
    skips_w = np.asarray(skips_w, np.float32)
    skips_b = np.asarray(skips_b, np.float32)
    bn_g = np.asarray(bn_g, np.float32)
    bn_b = np.asarray(bn_b, np.float32)
    lin9_w = np.asarray(lin9_w, np.float32)
    bn9_g = np.asarray(bn9_g, np.float32)
    bn9_b = np.asarray(bn9_b, np.float32)
    lin10_w = np.asarray(lin10_w, np.float32)
    lin10_b = np.asarray(lin10_b, np.float32)

    wlin_h = np.stack([_blockdiag(lins_w[i], G) for i in range(8)])
    wskip_h = np.stack([_blockdiag(skips_w[b], G) for b in range(4)])
    w9_h = _blockdiag(lin9_w, G)                       # [125, 50]
    w10t_h = np.zeros((50, 25), np.float32)
    for g in range(G):
        for j in range(2):
            w10t_h[2 * g + j, g] = lin10_w[0, j]
    ident_h = np.eye(128, dtype=np.float32)
    sel_h = np.zeros((125, 5), np.float32)
    for g in range(G):
        for k in range(5):
            sel_h[5 * g + k, k] = 1.0
    sel9_h = np.zeros((50, 2), np.float32)
    for g in range(G):
        for j in range(2):
            sel9_h[2 * g + j, j] = 1.0
    bs125_h = np.zeros((125, 4), np.float32)
    for g in range(G):
        bs125_h[5 * g:5 * g + 5, :] = skips_b.T
    common = {
        "wlin": wlin_h,
        "wskip": wskip_h,
        "w9": w9_h,
        "w10t": w10t_h,
        "ident": ident_h,
        "sel": sel_h,
        "sel2": np.ascontiguousarray(sel_h.T),
        "sel9": sel9_h,
        "sel92": np.ascontiguousarray(sel9_h.T),
        "onesrow": np.ones((1, 125), np.float32),
        "pmask": (np.arange(125) < 5 * LASTG).astype(np.float32).reshape(125, 1),
        "pmask9": (np.arange(50) < 2 * LASTG).astype(np.float32).reshape(50, 1),
        "gmat": np.ascontiguousarray(bn_g.T),
        "bmat": np.ascontiguousarray(bn_b.T),
        "g9": np.ascontiguousarray(bn9_g.reshape(2, 1)),
        "b9": np.ascontiguousarray(bn9_b.reshape(2, 1)),
        "bs125": bs125_h,
        "b10rep": np.full((128, 1), float(lin10_b[0]), np.float32),
    }
    in_maps = []
    for c in range(N_CORES):
        m = dict(common)
        m["x_shard"] = np.ascontiguousarray(x[c * R:(c + 1) * R])
        in_maps.append(m)
    return in_maps


def kernel(**inputs):
    nc = _get_program(reps=1)
    in_maps = make_in_maps(**inputs)
    res = run_bass_kernel_spmd(nc, in_maps, core_ids=list(range(N_CORES)),
                               **_PROGRAM_CACHE.get("run_kwargs", {}))
    _PROGRAM_CACHE["last_results"] = res
    out = np.concatenate([res.results[c]["out"] for c in range(N_CORES)],
                         axis=0)
    return np.ascontiguousarray(out, dtype=np.float32)
